# revision 6
# baseline (speedup 1.0000x reference)
"""MobileMamba block kernel v2 for 8x Trainium2 NeuronCores.

Math restructure of the reference (identical to v1):
  xc   = silu(x @ w1.T + b1)                          # [E, L]
  c    = depthwise_conv5(xc) (+bd, BN affine folded)  # [E, L]
  xl   = silu(c)
  scalar first-order recurrence (B/C constant):
    g[e,t] = expA[e]*g[e,t-1] + xl[e,t]
    gp     = (CB/Dv)*g + xl,   CB = sum_s Bm*Cm
  out  = gp @ w2dv.T (+ b2), w2dv = w2.T * Dv

v2 changes vs baseline:
  - lc-major software pipeline (mm1(s) | conv/silu2(s-1) | scan/fold(s-1)
    | mm2(s-2)) so PE starts immediately and all engines stay overlapped.
  - diag tap matrices + aexp broadcasts built on host (one const DMA)
    instead of GpSimd affine_select/casts (frees ~13us of setup).
  - fold = tensor_scalar (4x) + tensor_tensor (2x) instead of the 1x
    scalar_tensor_tensor.
  - out copy PSUM->SBUF bf16 split between Scalar (activation+bias) and
    GpSimd (tensor_scalar+bias); output DMA'd as bf16, host upcasts.
  - optional: edge tap +2 prewritten into conv PSUM by DVE/GpSimd
    (PE_TAPS=4) so PE runs 4 matmuls per conv chunk instead of 5.
"""

import sys

for _p in ('/opt/trn_rl_repo',):
    if _p not in sys.path:
        sys.path.append(_p)

import numpy as np

import concourse.bass as bass
import concourse.tile as tile
from concourse import mybir

D = 256      # model dim
E = 512      # expanded dim
L = 2048     # sequence length
NCORES = 8
BN_EPS = 1e-5

F32 = mybir.dt.float32
BF16 = mybir.dt.bfloat16

EM = E // 128   # 4 channel tiles
DM = D // 128   # 2 model-dim tiles
CH = 512
LC = L // CH

# ---- tunables ----
PE_TAPS = 5           # 5: all taps on PE; 4: tap +2 prewritten by DVE
SCAN_SPLIT = {0: 'v', 1: 'v', 2: 'v', 3: 'v'}   # m-tile -> engine (v only)
COPY_ENG = ['s', 's', 's', 's', 's', 's', 's', 's']  # per (dt,lc) copy engine
# note: GpSimd cannot access PSUM (BIR verifier) -> 'g' is not usable here

# md1 mega-tensor column layout (bf16):
#   w1t: DM*512 | w2dv: EM*256 | zeros: 128
# (diag tap matrices are built on-device by GpSimd affine_select -- they are
#  99% zeros, so DMAing them wastes startup DMA bandwidth)
W1_0 = 0
W2_0 = W1_0 + DM * 512
Z_0 = W2_0 + EM * 256
MD1_COLS = Z_0 + 128

# mp param-table columns (f32) per m; then b2 (DM cols)
PT_B1 = 0
PT_CBIAS = 1
PT_CBDV = 2      # CB/Dv
PT_WP2 = 3       # conv tap +2 weight (PE_TAPS==4 path)
PT_EXPA = 4      # a
PT_EXPA2 = 5     # a^2 (decimated-scan decay)
PT_ACBDV = 6     # a*CB/Dv
PT_CBDV1 = 7     # 1 + CB/Dv
PT_TAPS = 8      # conv taps 0..4 (device-side diag build)
PT_NCOL = 13
MP_COLS = EM * PT_NCOL + DM

TAPS = (-2, -1, 0, 1, 2)


def _bcast(col_ap, n):
    return bass.AP(tensor=col_ap.tensor, offset=col_ap.offset,
                   ap=[col_ap.ap[0], [0, n]])


def build_nc(wsplit=True):
    nc = bass.Bass()
    xt = nc.declare_dram_parameter("xt", [D, L], BF16, isOutput=False)
    md1 = nc.declare_dram_parameter("md1", [128, MD1_COLS], BF16,
                                    isOutput=False)
    mp = nc.declare_dram_parameter("mp", [128, MP_COLS], F32, isOutput=False)
    outT = nc.declare_dram_parameter("outT", [D, L], BF16, isOutput=True)

    pe_taps = TAPS if PE_TAPS == 5 else TAPS[:4]

    with tile.TileContext(nc) as tc:
        with (
            tc.tile_pool(name="const", bufs=1) as const,
            tc.tile_pool(name="acts", bufs=1) as acts,
            tc.tile_pool(name="psA", bufs=3, space="PSUM") as psA,
            tc.tile_pool(name="psB", bufs=3, space="PSUM") as psB,
            tc.tile_pool(name="psC", bufs=2, space="PSUM") as psC,
        ):
            # ---- x + constants; dispatch order/engines tuned for startup.
            # Each dma_start costs ~700ns on the issuing sequencer, so the
            # first-needed tensors go first, split across Sync and GpSimd.
            mw_t = const.tile([128, MD1_COLS], BF16)
            xts2 = acts.tile([128, DM * L], BF16, name="xts2", tag="xts2")
            xts = [xts2[:, k * L:(k + 1) * L] for k in range(DM)]

            def dma_x(eng, c0, c1):
                # one 3D-AP dma for both k-tiles, cols [c0:c1)
                ob = xts2[:, c0:c1]
                out_ap = bass.AP(tensor=ob.tensor, offset=ob.offset,
                                 ap=[ob.ap[0], [L, DM], [1, c1 - c0]])
                ib = xt[0:128, c0:c1]
                in_ap = bass.AP(tensor=ib.tensor, offset=ib.offset,
                                ap=[ib.ap[0], [128 * L, DM], [1, c1 - c0]])
                eng.dma_start(out=out_ap, in_=in_ap)

            # Startup is input-DMA-bandwidth bound: keep the first-needed
            # 0.5MB (xt chunk 0 + w1) alone on the fast Sync ring, mp on
            # Scalar, and let the 768KB xt remainder trickle through the
            # slow GpSimd ring (needed only from step 1 onward).
            # PE warm-up: dependency-free matmuls on a memset tile fill
            # [~5us, ~12us] so the HAM clock-gate is warm (and stays warm)
            # when the first data-dependent matmul issues at ~14us. Sized to
            # finish before data arrives so it never delays real work.
            dummy = const.tile([128, CH], BF16)
            nc.vector.memset(dummy, 0.0)
            psw0 = psA.tile([128, CH], F32, name="ps1", tag="ps1")
            for _ in range(22):
                nc.tensor.matmul(out=psw0, lhsT=dummy[:, 0:128],
                                 rhs=dummy, start=True, stop=True)

            mp_t = const.tile([128, MP_COLS], F32)
            dma_x(nc.sync, 0, CH)
            nc.sync.dma_start(out=mw_t[:, 0:W2_0], in_=md1[:, 0:W2_0])
            nc.scalar.dma_start(out=mp_t, in_=mp[:, :])
            dma_x(nc.scalar, CH, 2 * CH)
            dma_x(nc.gpsimd, 2 * CH, L)
            nc.scalar.dma_start(out=mw_t[:, W2_0:], in_=md1[:, W2_0:])

            # ---- constant slices ----
            w1s = [mw_t[:, W1_0 + k * 512:W1_0 + (k + 1) * 512]
                   for k in range(DM)]
            # diag tap matrices built by GpSimd (idle early): ~300ns each
            diag = [[None] * 5 for _ in range(EM)]
            for m in range(EM):
                for j in range(5):
                    dg = const.tile([128, 128], BF16, name=f"dg{m}_{j}",
                                    tag=f"dg{m}_{j}")
                    tc0 = m * PT_NCOL + PT_TAPS + j
                    nc.gpsimd.affine_select(
                        out=dg, in_=_bcast(mp_t[:, tc0:tc0 + 1], 128),
                        pattern=[[1, 128]], base=0, channel_multiplier=-1,
                        compare_op=mybir.AluOpType.is_equal, fill=0.0)
                    diag[m][j] = dg
            w2dvs = [mw_t[:, W2_0 + ec * 256:W2_0 + (ec + 1) * 256]
                     for ec in range(EM)]
            zl = mw_t[:, Z_0:Z_0 + 128]
            pts = [mp_t[:, m * PT_NCOL:(m + 1) * PT_NCOL] for m in range(EM)]
            b2s = [mp_t[:, EM * PT_NCOL + dt_:EM * PT_NCOL + dt_ + 1]
                   for dt_ in range(DM)]

            xc = [acts.tile([128, L], BF16, name=f"xc{m}", tag=f"xc{m}")
                  for m in range(EM)]
            xl = [acts.tile([128, L], BF16, name=f"xl{m}", tag=f"xl{m}")
                  for m in range(EM)]
            g = [acts.tile([128, L], BF16, name=f"g{m}", tag=f"g{m}")
                 for m in range(EM)]
            gp = [acts.tile([128, L], BF16, name=f"gp{m}", tag=f"gp{m}")
                  for m in range(EM)]
            tmp = [acts.tile([128, CH], BF16, name=f"tmp{m}", tag=f"tmp{m}")
                   for m in range(EM)]
            osb = [acts.tile([128, L], BF16, name=f"o{dt_}", tag=f"o{dt_}")
                   for dt_ in range(DM)]

            # ---- stages ----
            def mm1_stage(m, lc):
                c0, c1 = lc * CH, (lc + 1) * CH
                ps1 = psA.tile([128, CH], F32, name="ps1", tag="ps1")
                for k in range(DM):
                    nc.tensor.matmul(
                        out=ps1, lhsT=w1s[k][:, m * 128:(m + 1) * 128],
                        rhs=xts[k][:, c0:c1],
                        start=(k == 0), stop=(k == DM - 1))
                nc.scalar.activation(
                    out=xc[m][:, c0:c1], in_=ps1,
                    func=mybir.ActivationFunctionType.Silu,
                    bias=pts[m][:, PT_B1:PT_B1 + 1], scale=1.0)

            def conv_stage(m, a0, b0):
                ps2 = psB.tile([128, b0 - a0], F32, name="ps2", tag="ps2")
                if PE_TAPS == 4:
                    # DVE prewrites tap +2 (w4 * xc[t+2]); PE accumulates
                    # the other 4 taps on top (start=False).
                    hi = min(b0, L - 2)
                    nc.vector.tensor_scalar(
                        out=ps2[:, 0:hi - a0],
                        in0=xc[m][:, a0 + 2:hi + 2],
                        scalar1=pts[m][:, PT_WP2:PT_WP2 + 1], scalar2=None,
                        op0=mybir.AluOpType.mult)
                    if hi < b0:
                        nc.vector.memset(ps2[:, hi - a0:b0 - a0], 0.0)
                first = PE_TAPS == 5
                for j, dlt in enumerate(pe_taps):
                    lo, hi = max(0, -dlt), L - max(0, dlt)
                    a, b_ = max(a0, lo), min(b0, hi)
                    if a >= b_:
                        continue
                    nc.tensor.matmul(
                        out=ps2[:, a - a0:b_ - a0],
                        lhsT=diag[m][dlt + 2],
                        rhs=xc[m][:, a + dlt:b_ + dlt],
                        start=(first and j == 0), stop=(j == len(pe_taps) - 1),
                        skip_group_check=True)
                nc.scalar.activation(
                    out=xl[m][:, a0:b0], in_=ps2,
                    func=mybir.ActivationFunctionType.Silu,
                    bias=pts[m][:, PT_CBIAS:PT_CBIAS + 1], scale=1.0)

            def scan_stage(m, a0, b0):
                pe_col = pts[m][:, PT_EXPA:PT_EXPA + 1]
                nc.vector.tensor_tensor_scan(
                    out=g[m][:, a0:b0], data0=_bcast(pe_col, b0 - a0),
                    data1=xl[m][:, a0:b0],
                    initial=(0.0 if a0 == 0 else g[m][:, a0 - 1:a0]),
                    op0=mybir.AluOpType.mult, op1=mybir.AluOpType.add)

            def fold_stage(m, a0, b0):
                n = b0 - a0
                nc.vector.tensor_scalar(
                    out=tmp[m][:, 0:n], in0=g[m][:, a0:b0],
                    scalar1=pts[m][:, PT_CBDV:PT_CBDV + 1], scalar2=None,
                    op0=mybir.AluOpType.mult)
                nc.vector.tensor_tensor(
                    out=gp[m][:, a0:b0], in0=tmp[m][:, 0:n],
                    in1=xl[m][:, a0:b0], op=mybir.AluOpType.add)

            def mm2_stage(dt_, a0, b0, ceng):
                ps3 = psC.tile([128, CH], F32, name="ps3", tag="ps3")
                n = b0 - a0
                for ec in range(EM):
                    nc.tensor.matmul(
                        out=ps3[:, 0:n],
                        lhsT=w2dvs[ec][:, dt_ * 128:(dt_ + 1) * 128],
                        rhs=gp[ec][:, a0:b0],
                        start=(ec == 0), stop=(ec == EM - 1))
                if ceng == 's':
                    nc.scalar.activation(
                        out=osb[dt_][:, a0:b0], in_=ps3[:, 0:n],
                        func=mybir.ActivationFunctionType.Identity,
                        bias=b2s[dt_], scale=1.0)
                else:
                    nc.vector.tensor_scalar(
                        out=osb[dt_][:, a0:b0], in0=ps3[:, 0:n],
                        scalar1=b2s[dt_], scalar2=None,
                        op0=mybir.AluOpType.add)
                nc.sync.dma_start(
                    out=outT[dt_ * 128:(dt_ + 1) * 128, a0:b0],
                    in_=osb[dt_][:, a0:b0])


            # ---- warm psB banks so has_written bits are set before the
            # first start=False accumulation group (PE_TAPS==4 path).
            if PE_TAPS == 4:
                for _ in range(3):
                    psw = psB.tile([128, CH], F32, name="ps2", tag="ps2")
                    nc.tensor.matmul(out=psw, lhsT=zl, rhs=mw_t[:, 0:CH],
                                     start=True, stop=True)

            # ---- software pipeline, lc-major (v2.1 form): mm1(s) |
            # conv/scan/fold(s-1) | mm2(s-2); last mm2 chunk split in halves.
            def mm2_slots(s):
                lc = s - 2
                if not (0 <= lc < LC):
                    return []
                if lc < LC - 1:
                    return [(dt_, lc * CH, (lc + 1) * CH, 's')
                            for dt_ in range(DM)]
                return [(dt_, lc * CH + s0, lc * CH + s1, 's')
                        for (s0, s1) in ((0, 256), (256, 512))
                        for dt_ in range(DM)]

            def tail_units(lc):
                a0 = lc * CH
                if lc < LC - 1:
                    return [(a0, a0 + CH)]
                return [(a0, a0 + CH // 2), (a0 + CH // 2, a0 + CH)]

            for s in range(LC + 2):
                if s < LC:
                    for m in range(EM):
                        mm1_stage(m, s)
                if 1 <= s <= LC:
                    for (a0, b0) in tail_units(s - 1):
                        for m in range(EM):
                            conv_stage(m, a0, b0)
                            scan_stage(m, a0, b0)
                            fold_stage(m, a0, b0)
                for sl in mm2_slots(s):
                    mm2_stage(*sl)

    if wsplit:
        _split_waits(nc)
    return nc


_WSPLIT_SKIP = ("InstAllEngineBarrier", "InstNoOp",
                "InstEventSemaphore", "InstUnconditionalBranch")


def _split_waits(nc, max_waits=1):
    """Walrus codegen allows a single sync-wait command per TPB instruction.

    Move all-but-one waits of any over-limit instruction onto preceding
    NoOps (one wait each) on the same engine; same-engine program order
    makes this sound.
    """
    n_split = 0
    for f in nc.m.functions:
        for bb in f.blocks:
            out = []
            for inst in bb.instructions:
                si = inst.sync_info
                waits = list(si.on_wait) if si and si.on_wait else []
                if (len(waits) > max_waits
                        and inst.__class__.__name__ not in _WSPLIT_SKIP):
                    spill, keep = waits[:-max_waits], waits[-max_waits:]
                    for i, w in enumerate(spill):
                        out.append(mybir.InstNoOp(
                            name=f"{inst.name}_ws{i}",
                            engine=inst.engine,
                            sync_info=mybir.SyncInfo(on_wait=[w],
                                                     on_update=[]),
                        ))
                        n_split += 1
                    si.on_wait = keep
                out.append(inst)
            if n_split:
                bb.instructions = out
    return nc


def _to_bf16(a):
    import ml_dtypes
    return np.asarray(a, np.float32).astype(ml_dtypes.bfloat16)


def host_params(w1, b1, wd, bd, gamma, beta, rmean, rvar, A, Bm, Cm, Dv,
                w2, b2):
    s = (gamma / np.sqrt(rvar + BN_EPS)).astype(np.float32)
    cw = (wd[:, 0, :] * s[:, None]).astype(np.float32)            # [E, 5]
    cbias = (bd * s + beta - rmean * s).astype(np.float32)        # [E]
    expA = np.exp(A).astype(np.float32)                           # [E]
    CB = (Bm * Cm).sum(1).astype(np.float32)                      # [E]
    w1t = np.asarray(w1, np.float32).T                            # [D, E]
    w2t = np.asarray(w2, np.float32).T                            # [E, D]

    dv = np.asarray(Dv, np.float32).copy()
    tiny = np.abs(dv) < 1e-6
    dv[tiny] = np.where(dv[tiny] < 0, -1e-6, 1e-6)
    cbdv = CB / dv

    md1 = np.zeros((128, MD1_COLS), np.float32)
    for k in range(DM):
        md1[:, W1_0 + k * 512:W1_0 + (k + 1) * 512] = \
            w1t[k * 128:(k + 1) * 128, :]
    for ec in range(EM):
        blk = w2t[ec * 128:(ec + 1) * 128, :]
        md1[:, W2_0 + ec * 256:W2_0 + (ec + 1) * 256] = \
            blk * dv[ec * 128:(ec + 1) * 128, None]
    md1 = _to_bf16(md1)

    mpm = np.zeros((128, MP_COLS), np.float32)
    for m in range(EM):
        sl = slice(m * 128, (m + 1) * 128)
        mpm[:, m * PT_NCOL + PT_B1] = np.asarray(b1, np.float32)[sl]
        mpm[:, m * PT_NCOL + PT_CBIAS] = cbias[sl]
        mpm[:, m * PT_NCOL + PT_CBDV] = cbdv[sl]
        mpm[:, m * PT_NCOL + PT_WP2] = cw[sl, 4]
        mpm[:, m * PT_NCOL + PT_EXPA] = expA[sl]
        mpm[:, m * PT_NCOL + PT_EXPA2] = expA[sl] ** 2
        mpm[:, m * PT_NCOL + PT_ACBDV] = expA[sl] * cbdv[sl]
        mpm[:, m * PT_NCOL + PT_CBDV1] = 1.0 + cbdv[sl]
        for j in range(5):
            mpm[:, m * PT_NCOL + PT_TAPS + j] = cw[sl, j]
    for dt_ in range(DM):
        mpm[:, EM * PT_NCOL + dt_] = \
            np.asarray(b2, np.float32)[dt_ * 128:(dt_ + 1) * 128]

    return dict(md1=md1, mp=mpm)


_CACHED_NC = None


def kernel(x, w1, b1, wd, bd, gamma, beta, rmean, rvar, A, Bm, Cm, Dv, w2, b2,
           **run_kwargs):
    from concourse.bass_utils import run_bass_kernel_spmd
    global _CACHED_NC
    if _CACHED_NC is None:
        _CACHED_NC = build_nc()
    nc = _CACHED_NC

    params = host_params(w1, b1, wd, bd, gamma, beta, rmean, rvar,
                         A, Bm, Cm, Dv, w2, b2)
    x = np.asarray(x, dtype=np.float32)
    in_maps = []
    for i in range(NCORES):
        m = dict(params)
        m["xt"] = _to_bf16(np.ascontiguousarray(x[i].T))  # [D, L] bf16
        in_maps.append(m)

    res = run_bass_kernel_spmd(nc, in_maps, core_ids=list(range(NCORES)),
                               **run_kwargs)
    out = np.stack([np.asarray(r["outT"], np.float32).T
                    for r in res.results])  # [B, L, D]
    if run_kwargs:
        kernel.last_result = res
    return out
